# revision 1
# baseline (speedup 1.0000x reference)
"""Trainium2 Bass kernel for the Exprnn-style model (nn_Exprnn_2542620639651).

Pipeline: enc MLP (2x relu) -> orthogonal RNN with modrelu over T=512 ->
linear decoder.  Sharding: pure data parallel over batch (8 cores x 1024).

Instead of 512 serial matmul steps, the recurrence is solved by a
fixed-point linear-scan decomposition.  modrelu(z) = z + d(z) with
|d| <= |mb| = 0.01 always, so  h_t = sum_{k<=t} (u_k + d_k) R^{t-k}  is a
LINEAR scan over v = u + d plus a tiny correction stream d:

  scan 1:  h~_t = linear_scan(u)            (d = 0)
  extract: dd_t = -(modrelu(h~_t) - h~_t)   (parallel elementwise)
  scan 2:  out  = decode(linear_scan(u + d))

Each scan runs as 57 blocks of TB=9 timesteps (T padded 512->513).  Time
lives on SBUF partitions (10j+r for timestep-in-block j, hidden r), batch
(1024) on the free dim.  A block is ONE triangular block matmul with
constant weights  Win @ R^(j-k)  (+ a carry matmul  R^(j+1)  against the
previous block's last state, + a dd matmul in scan 2), all accumulated in
PSUM rows 0..89; rows 96..105 hold the carry (h at block end) produced by
extra lhsT columns, evicted with a partition-shifting copy to SBUF rows
0..9 for the next block's carry matmul.  The decoder (W3@W4) is folded
into scan 2's weights.  The only serial dependency left is the 57-step
carry chain per scan.

Validated end-to-end vs the fp32 reference at ~4e-3 max rel err with
realistic dtypes (bf16 x2/dd/A/B weights, f32r carry matmuls, fp32 PSUM).
"""

import os
import sys
from contextlib import ExitStack

for _p in ("/root/.axon_site/_ro/trn_rl_repo", "/opt/trn_rl_repo"):
    if os.path.isdir(_p) and _p not in sys.path:
        sys.path.append(_p)

import numpy as np
import ml_dtypes

import concourse.bass as bass
import concourse.tile as tile
from concourse import bacc, mybir
from concourse.bass_utils import run_bass_kernel_spmd

dt = mybir.dt
Alu = mybir.AluOpType
Act = mybir.ActivationFunctionType

# Problem shape (hardcoded per contract)
B, T, NI, H = 8192, 512, 2, 10
NCORES = 8
NB = B // NCORES          # 1024 batch per core = free dim
TB = 9                    # timesteps per scan block
NBLK = 57                 # blocks (57*9 = 513, time padded with zeros)
TPAD = TB * NBLK
KA = 10 * TB              # 90: x2/dd contraction partitions (outputs rows 0..89)
CO = 96                   # carry-row base in PSUM (32-aligned read); evicted to SBUF rows 0..9
M = CO + 10               # 106: psum rows = outputs(0:90) + pad + carry(96:106)
KX = NI * TB              # 12: encoder-input partitions
S = 2                     # column streams (matmul moving dim = NB/S = 512)
NS = NB // S
KBIG = float(2.0 ** 40)

_cache = {}


def _build_program():
    nc = bacc.Bacc("TRN2", target_bir_lowering=False, debug=False)
    f32, f32r, bf16 = dt.float32, dt.float32r, dt.bfloat16
    global bf16_

    bf16_ = bf16
    xin = nc.dram_tensor("xin", [NBLK, KA, NB], bf16_, kind="ExternalInput").ap()
    dlw2 = nc.dram_tensor("lw2", [KA, KA], bf16_, kind="ExternalInput").ap()
    da1 = nc.dram_tensor("a1", [KA, M], bf16, kind="ExternalInput").ap()
    da2 = nc.dram_tensor("a2", [KA, M], bf16, kind="ExternalInput").ap()
    db2w = nc.dram_tensor("b2w", [KA, M], bf16, kind="ExternalInput").ap()
    dc1 = nc.dram_tensor("c1w", [10, M], f32r, kind="ExternalInput").ap()
    dc2 = nc.dram_tensor("c2w", [10, M], f32r, kind="ExternalInput").ap()
    db2t = nc.dram_tensor("b2t", [KA, 1], f32, kind="ExternalInput").ap()
    dcmul = nc.dram_tensor("cmul", [KA, 1], f32, kind="ExternalInput").ap()
    dchi = nc.dram_tensor("chi", [KA, 1], f32, kind="ExternalInput").ap()
    dclo = nc.dram_tensor("clo", [KA, 1], f32, kind="ExternalInput").ap()
    yout = nc.dram_tensor("yout", [NBLK, KA, NB], f32, kind="ExternalOutput").ap()

    with tile.TileContext(nc) as tc, ExitStack() as ctx:
        wp = ctx.enter_context(tc.tile_pool(name="weights", bufs=1))
        xp = ctx.enter_context(tc.tile_pool(name="xin", bufs=3))
        x2p = ctx.enter_context(tc.tile_pool(name="x2", bufs=6))
        zp = ctx.enter_context(tc.tile_pool(name="zt", bufs=3))
        ep = ctx.enter_context(tc.tile_pool(name="et", bufs=3))
        ddp = ctx.enter_context(tc.tile_pool(name="dd", bufs=4))
        c1p = ctx.enter_context(tc.tile_pool(name="car1", bufs=2))
        c2p = ctx.enter_context(tc.tile_pool(name="car2", bufs=2))
        otp = ctx.enter_context(tc.tile_pool(name="ot", bufs=3))
        sps = ctx.enter_context(tc.tile_pool(name="scanps", bufs=4, space="PSUM"))

        def wtile(name, dram, shape, dtype, rows=None):
            t = wp.tile(shape, dtype, tag=name)
            nc.sync.dma_start(t[rows, :] if rows else t[:], dram[:])
            return t

        lw2 = wtile("lw2", dlw2, [KA, KA], bf16)
        a1 = wtile("a1", da1, [KA, M], bf16)
        a2 = wtile("a2", da2, [KA, M], bf16)
        b2w = wtile("b2w", db2w, [KA, M], bf16)
        c1w = wtile("c1w", dc1, [10, M], f32r)
        c2w = wtile("c2w", dc2, [10, M], f32r)
        b2t = wtile("b2t", db2t, [KA, 1], f32)
        cmul = wtile("cmul", dcmul, [KA, 1], f32)
        chi = wtile("chi", dchi, [KA, 1], f32)
        clo = wtile("clo", dclo, [KA, 1], f32)

        car1 = car2 = None
        NH = NB // 2
        for b in range(NBLK):
            # ---- encoder layer 2 (enc1 folded into host prep) ----
            xt = xp.tile([KA, NB], bf16)
            nc.sync.dma_start(xt[:], xin[b])
            x2t = x2p.tile([KA, NB], bf16)
            ps = sps.tile([M, NB], f32, tag="scan")
            nc.tensor.matmul(ps[:KA, :NH], lw2[:], xt[:, :NH], start=True, stop=True)
            nc.tensor.matmul(ps[:KA, NH:], lw2[:], xt[:, NH:], start=True, stop=True)
            nc.scalar.activation(x2t[:], ps[:KA, :], Act.Relu, bias=b2t[:])

            # ---- scan 1: h~ block + carry chain ----
            zt = zp.tile([KA, NB], bf16)
            ncar1 = c1p.tile([10, NB], f32r)
            ps = sps.tile([M, NB], f32, tag="scan")
            nc.tensor.matmul(ps[:, :NH], a1[:], x2t[:, :NH], start=True, stop=(b == 0))
            nc.tensor.matmul(ps[:, NH:], a1[:], x2t[:, NH:], start=True, stop=(b == 0))
            if b > 0:
                nc.tensor.matmul(ps[:, :NH], c1w[:], car1[:, :NH],
                                 start=False, stop=True, skip_group_check=True)
                nc.tensor.matmul(ps[:, NH:], c1w[:], car1[:, NH:],
                                 start=False, stop=True, skip_group_check=True)
            # z~ eviction (bf16) on ACT; carry eviction shifted to rows 0..9 on DVE
            nc.scalar.activation(zt[:], ps[:KA, :], Act.Copy)
            nc.vector.tensor_copy(ncar1[:, :NH], ps[CO:M, :NH])
            nc.scalar.activation(ncar1[:, NH:], ps[CO:M, NH:], Act.Copy)
            car1 = ncar1

            # ---- dd extraction on DVE (bf16 4x mode) ----
            # dd_neg = max(min(z*c, |mb|), -|mb|)   (c = 1 or -2^40 per row)
            et = ep.tile([KA, NB], bf16)
            ddt = ddp.tile([KA, NB], bf16)
            nc.vector.tensor_scalar(et[:], zt[:], cmul[:], chi[:],
                                    Alu.mult, Alu.min)
            nc.vector.tensor_scalar(ddt[:], et[:], clo[:], None, Alu.max)

            # ---- scan 2: decoded output + its own carry chain ----
            ot = otp.tile([KA, NB], f32)
            ncar2 = c2p.tile([10, NB], f32r)
            ps = sps.tile([M, NB], f32, tag="scan")
            nc.tensor.matmul(ps[:, :NH], a2[:], x2t[:, :NH], start=True, stop=False)
            nc.tensor.matmul(ps[:, NH:], a2[:], x2t[:, NH:], start=True, stop=False)
            nc.tensor.matmul(ps[:, :NH], b2w[:], ddt[:, :NH], start=False, stop=(b == 0))
            nc.tensor.matmul(ps[:, NH:], b2w[:], ddt[:, NH:], start=False, stop=(b == 0))
            if b > 0:
                nc.tensor.matmul(ps[:, :NH], c2w[:], car2[:, :NH],
                                 start=False, stop=True, skip_group_check=True)
                nc.tensor.matmul(ps[:, NH:], c2w[:], car2[:, NH:],
                                 start=False, stop=True, skip_group_check=True)
            # output eviction on ACT; carry eviction on DVE
            nc.scalar.activation(ot[:], ps[:KA, :], Act.Copy)
            nc.vector.tensor_copy(ncar2[:, :NH], ps[CO:M, :NH])
            nc.scalar.activation(ncar2[:, NH:], ps[CO:M, NH:], Act.Copy)
            car2 = ncar2
            nc.sync.dma_start(yout[b], ot[:])

    nc.compile()
    return nc


def _prep_inputs(inputs):
    X = np.ascontiguousarray(inputs["X"], dtype=np.float32)
    W1, b1v, W2, b2v = (np.asarray(inputs[k], np.float64) for k in ("W1", "b1", "W2", "b2"))
    Win, R, mbv = (np.asarray(inputs[k], np.float64) for k in ("Win", "R", "mb"))
    W3, b3v, W4, b4v = (np.asarray(inputs[k], np.float64) for k in ("W3", "b3", "W4", "b4"))
    Dm = W3 @ W4
    c4 = (b3v @ W4 + b4v).astype(np.float32)

    Rp = [np.eye(H)]
    for _ in range(TB + 1):
        Rp.append(Rp[-1] @ R)

    def blockdiag(Mx, reps):
        K, Ho = Mx.shape
        out = np.zeros((K * reps, Ho * reps), np.float32)
        for i in range(reps):
            out[i * K:(i + 1) * K, i * Ho:(i + 1) * Ho] = Mx
        return out

    def lhsA(dec):
        L = np.zeros((KA, M), np.float64)
        for k in range(TB):
            for j in range(k, TB):
                blk = Win @ Rp[j - k]
                L[10 * k:10 * k + 10, 10 * j:10 * j + 10] = blk @ Dm if dec else blk
            L[10 * k:10 * k + 10, CO:] = Win @ Rp[TB - 1 - k]
        return L

    def lhsB(dec):
        L = np.zeros((KA, M), np.float64)
        for k in range(TB):
            for j in range(k, TB):
                blk = Rp[j - k]
                L[10 * k:10 * k + 10, 10 * j:10 * j + 10] = -(blk @ Dm) if dec else -blk
            L[10 * k:10 * k + 10, CO:] = -Rp[TB - 1 - k]
        return L

    def lhsC(dec):
        L = np.zeros((10, M), np.float64)
        for j in range(TB):
            blk = Rp[j + 1]
            L[:, 10 * j:10 * j + 10] = blk @ Dm if dec else blk
        L[:, CO:] = Rp[TB]
        return L

    # host enc1 (1% of model FLOPs): x1 = relu(X@W1+b1), zero-padded T -> TPAD,
    # reshaped to [core, block, 10j+r, n], bf16
    x1 = np.maximum(X @ W1.astype(np.float32) + b1v.astype(np.float32), 0)
    Xc = x1.reshape(NCORES, NB, T, H)
    Xp = np.zeros((NCORES, NB, TPAD, H), np.float32)
    Xp[:, :, :T] = Xc
    Xin = np.ascontiguousarray(
        Xp.reshape(NCORES, NB, NBLK, TB, H).transpose(0, 2, 3, 4, 1)
        .reshape(NCORES, NBLK, KA, NB).astype(ml_dtypes.bfloat16)
    )

    mbt = np.tile(mbv, TB).astype(np.float32)
    shared = {
        "lw2": blockdiag(W2, TB).astype(ml_dtypes.bfloat16),
        "a1": lhsA(False).astype(ml_dtypes.bfloat16),
        "a2": lhsA(True).astype(ml_dtypes.bfloat16),
        "b2w": lhsB(True).astype(ml_dtypes.bfloat16),
        "c1w": lhsC(False).astype(np.float32),
        "c2w": lhsC(True).astype(np.float32),
        "b2t": np.ascontiguousarray(np.tile(b2v, TB).astype(np.float32).reshape(KA, 1)),
        "cmul": np.ascontiguousarray(np.where(mbt <= 0, 1.0, -KBIG).astype(np.float32).reshape(KA, 1)),
        "chi": np.ascontiguousarray(np.abs(mbt).reshape(KA, 1)),
        "clo": np.ascontiguousarray((-np.abs(mbt)).reshape(KA, 1)),
    }
    in_maps = [dict(shared, xin=Xin[c]) for c in range(NCORES)]
    return in_maps, c4


def _gather(results, c4):
    out = np.empty((B, T, H), np.float32)
    for c in range(NCORES):
        yo = results[c]["yout"]  # [NBLK, KA, NB]
        full = yo.reshape(NBLK, TB, H, NB).transpose(3, 0, 1, 2).reshape(NB, TPAD, H)
        out[c * NB:(c + 1) * NB] = full[:, :T]
    if np.any(c4):
        out += c4
    return out


def kernel(**inputs):
    if "nc" not in _cache:
        _cache["nc"] = _build_program()
    in_maps, c4 = _prep_inputs(inputs)
    res = run_bass_kernel_spmd(_cache["nc"], in_maps, core_ids=list(range(NCORES)))
    return _gather(res.results, c4)



# revision 6
# speedup vs baseline: 1.0103x; 1.0103x over previous
"""Trainium2 Bass kernel for the Exprnn-style model (nn_Exprnn_2542620639651).

Pipeline: enc MLP (2x relu) -> orthogonal RNN with modrelu over T=512 ->
linear decoder.  Sharding: pure data parallel over batch (8 cores x 1024).

v2 design (vs baseline): the encoder MLP (<1 GFLOP) moves to host prep, so
the device solves only the recurrence via the fixed-point linear-scan
decomposition:  modrelu(z) = z + d(z),  d_{t,r} = sigma_r * clip(z~_t,r ,
+-|mb_r|)  (sigma_r = sign convention folded into the dd-scan weights),
where z~ = linear_scan(u) approximates the true pre-activation.

Per block of TB=9 timesteps (time on partitions 10j+r, batch 1024 on the
free dim, 57 blocks):
  ps1 = A1 @ x2  (+ C1 @ carry1)    triangular block matmul, K=90
  s1  = clip(ps1, +-|mb|)           ONE DVE tensor_scalar: eviction + full
                                    dd extraction; carry rows (96..105)
                                    pass through via +-1e30 clip bounds
  ps2 = A2 @ x2 + B2 @ s1[0:90] (+ C2 @ carry2)
  s2  = copy(ps2)                   ONE ACT eviction -> fp16 out + carry2
All matmuls contracting K=90 are split into K=64 (row strips 0-1) and K=26
(strip 2) chunks so the K=10 carry matmuls (tile_position row 96, strip 3)
run CONCURRENTLY in the PE array; carries stay at partitions 96..105 of
s1/s2 and are consumed directly as the next block's matmul rhs (no shift
copies).  Everything on-device is fp16 (inputs, weights, evictions, DMA
out); PSUM accumulates fp32.  Host converts fp16 out -> f32 and adds the
decoder bias.

Validated vs the fp32 reference at ~3.8e-3 max rel err in numpy simulation
with fp16 quantization at all the above points.
"""

import os
import sys
from contextlib import ExitStack

for _p in ("/root/.axon_site/_ro/trn_rl_repo", "/opt/trn_rl_repo"):
    if os.path.isdir(_p) and _p not in sys.path:
        sys.path.append(_p)

import numpy as np
import ml_dtypes

import concourse.bass as bass
import concourse.tile as tile
from concourse import bacc, mybir
from concourse.bass_utils import run_bass_kernel_spmd

dt = mybir.dt
Alu = mybir.AluOpType
Act = mybir.ActivationFunctionType

# Problem shape (hardcoded per contract)
B, T, NI, H = 8192, 512, 2, 10
NCORES = 8
NB = B // NCORES          # 1024 batch per core = free dim
TB = 9                    # timesteps per scan block
NBLK = 57                 # blocks (57*9 = 513, time padded with zeros)
TPAD = TB * NBLK
KA = 10 * TB              # 90: contraction partitions
CO = 96                   # carry-row base (32-aligned partition start)
M = CO + 10               # 106 psum rows: outputs 0..89, carry 96..105
NH = NB // 2              # 512 = PSUM bank limit per matmul
KS = 64                   # K-split point: strips 0-1 | strip 2

_cache = {}


def _build_program():
    nc = bacc.Bacc("TRN2", target_bir_lowering=False, debug=False)
    f32, f16 = dt.float32, dt.float16
    xin = nc.dram_tensor("xin", [NBLK, KA, NB], f16, kind="ExternalInput").ap()
    da1 = nc.dram_tensor("a1", [KA, M], f16, kind="ExternalInput").ap()
    da2 = nc.dram_tensor("a2", [KA, M], f16, kind="ExternalInput").ap()
    db2 = nc.dram_tensor("b2w", [KA, M], f16, kind="ExternalInput").ap()
    dc1 = nc.dram_tensor("c1w", [10, M], f16, kind="ExternalInput").ap()
    dc2 = nc.dram_tensor("c2w", [10, M], f16, kind="ExternalInput").ap()
    dchi = nc.dram_tensor("chi", [M, 1], f32, kind="ExternalInput").ap()
    dclo = nc.dram_tensor("clo", [M, 1], f32, kind="ExternalInput").ap()
    yout = nc.dram_tensor("yout", [NBLK, KA, NB], f16, kind="ExternalOutput").ap()

    with tile.TileContext(nc) as tc, ExitStack() as ctx:
        wp = ctx.enter_context(tc.tile_pool(name="weights", bufs=1))
        xp = ctx.enter_context(tc.tile_pool(name="xin", bufs=6))
        s1p = ctx.enter_context(tc.tile_pool(name="s1", bufs=4))
        s2p = ctx.enter_context(tc.tile_pool(name="s2", bufs=4))
        sps = ctx.enter_context(tc.tile_pool(name="ps", bufs=4, space="PSUM"))

        a1 = wp.tile([KA, M], f16, tag="a1")
        nc.sync.dma_start(a1[:], da1[:])
        a2 = wp.tile([KA, M], f16, tag="a2")
        nc.sync.dma_start(a2[:], da2[:])
        b2 = wp.tile([KA, M], f16, tag="b2")
        nc.sync.dma_start(b2[:], db2[:])
        # carry weights live on partitions 96..105 so the carry matmuls
        # occupy PE row strip 3, concurrent with the K-split strips 0-2
        c1 = wp.tile([M, M], f16, tag="c1")
        nc.sync.dma_start(c1[CO:M, :], dc1[:])
        c2 = wp.tile([M, M], f16, tag="c2")
        nc.sync.dma_start(c2[CO:M, :], dc2[:])
        chi = wp.tile([M, 1], f32, tag="chi")
        nc.sync.dma_start(chi[:], dchi[:])
        clo = wp.tile([M, 1], f32, tag="clo")
        nc.sync.dma_start(clo[:], dclo[:])

        s1prev = s2prev = None
        for b in range(NBLK):
            xt = xp.tile([KA, NB], f16)
            nc.sync.dma_start(xt[:], xin[b])

            ps1 = sps.tile([M, NB], f32, tag="ps")
            for lo, hi in ((0, NH), (NH, NB)):
                # carry matmul FIRST (start=True): a strip-3 matmul joining a
                # group with start=False crashes the device (HW quirk), but
                # leading the group works and is semantically identical
                if b > 0:
                    nc.tensor.matmul(ps1[:, lo:hi], c1[CO:M, :],
                                     s1prev[CO:M, lo:hi],
                                     start=True, stop=False,
                                     tile_position=(CO, 0))
                nc.tensor.matmul(ps1[:, lo:hi], a1[:KS, :], xt[:KS, lo:hi],
                                 start=(b == 0), stop=False)
                nc.tensor.matmul(ps1[:, lo:hi], a1[KS:, :], xt[KS:, lo:hi],
                                 start=False, stop=True)
            # eviction == dd extraction == carry handoff, one op per half
            s1 = s1p.tile([M, NB], f16)
            nc.vector.tensor_scalar(s1[:, :NH], ps1[:, :NH], chi[:], clo[:],
                                    Alu.min, Alu.max)
            nc.vector.tensor_scalar(s1[:, NH:], ps1[:, NH:], chi[:], clo[:],
                                    Alu.min, Alu.max)

            ps2 = sps.tile([M, NB], f32, tag="ps")
            for lo, hi in ((0, NH), (NH, NB)):
                if b > 0:
                    nc.tensor.matmul(ps2[:, lo:hi], c2[CO:M, :],
                                     s2prev[CO:M, lo:hi],
                                     start=True, stop=False,
                                     tile_position=(CO, 0))
                nc.tensor.matmul(ps2[:, lo:hi], a2[:KS, :], xt[:KS, lo:hi],
                                 start=(b == 0), stop=False)
                nc.tensor.matmul(ps2[:, lo:hi], a2[KS:, :], xt[KS:, lo:hi],
                                 start=False, stop=False)
                nc.tensor.matmul(ps2[:, lo:hi], b2[:KS, :], s1[:KS, lo:hi],
                                 start=False, stop=False)
                nc.tensor.matmul(ps2[:, lo:hi], b2[KS:, :], s1[KS:KA, lo:hi],
                                 start=False, stop=True)
            s2 = s2p.tile([M, NB], f16)
            nc.scalar.activation(s2[:, :NH], ps2[:, :NH], Act.Copy)
            nc.scalar.activation(s2[:, NH:], ps2[:, NH:], Act.Copy)
            nc.sync.dma_start(yout[b], s2[:KA, :])
            s1prev, s2prev = s1, s2

    nc.compile()
    return nc


def _prep_inputs(inputs):
    X = np.ascontiguousarray(inputs["X"], dtype=np.float32)
    W1, b1v, W2, b2v = (np.asarray(inputs[k], np.float64) for k in ("W1", "b1", "W2", "b2"))
    Win, R, mbv = (np.asarray(inputs[k], np.float64) for k in ("Win", "R", "mb"))
    W3, b3v, W4, b4v = (np.asarray(inputs[k], np.float64) for k in ("W3", "b3", "W4", "b4"))
    Dm = W3 @ W4
    c4 = (b3v @ W4 + b4v).astype(np.float32)

    Rp = [np.eye(H)]
    for _ in range(TB + 1):
        Rp.append(Rp[-1] @ R)

    sig = np.where(mbv <= 0, -1.0, 1.0)
    absmb = np.abs(mbv)

    def lhsA(dec):
        L = np.zeros((KA, M), np.float64)
        for k in range(TB):
            for j in range(k, TB):
                blk = Win @ Rp[j - k]
                L[10 * k:10 * k + 10, 10 * j:10 * j + 10] = blk @ Dm if dec else blk
            L[10 * k:10 * k + 10, CO:] = Win @ Rp[TB - 1 - k]
        return L

    def lhsB():
        L = np.zeros((KA, M), np.float64)
        for k in range(TB):
            for j in range(k, TB):
                L[10 * k:10 * k + 10, 10 * j:10 * j + 10] = np.diag(sig) @ Rp[j - k] @ Dm
            L[10 * k:10 * k + 10, CO:] = np.diag(sig) @ Rp[TB - 1 - k]
        return L

    def lhsC(dec):
        L = np.zeros((10, M), np.float64)
        for j in range(TB):
            blk = Rp[j + 1]
            L[:, 10 * j:10 * j + 10] = blk @ Dm if dec else blk
        L[:, CO:] = Rp[TB]
        return L

    # host encoder (<1 GFLOP): x2 = relu(relu(X@W1+b1)@W2+b2),
    # zero-padded T -> TPAD, reshaped to [core, block, 10j+r, n], fp16
    x1 = np.maximum(X @ W1.astype(np.float32) + b1v.astype(np.float32), 0)
    x2 = np.maximum(x1 @ W2.astype(np.float32) + b2v.astype(np.float32), 0)
    Xc = x2.reshape(NCORES, NB, T, H)
    Xp = np.zeros((NCORES, NB, TPAD, H), np.float32)
    Xp[:, :, :T] = Xc
    Xin = np.ascontiguousarray(
        Xp.reshape(NCORES, NB, NBLK, TB, H).transpose(0, 2, 3, 4, 1)
        .reshape(NCORES, NBLK, KA, NB).astype(np.float16)
    )

    chiv = np.concatenate([np.tile(absmb, TB), np.full(M - KA, 1e30)]).astype(np.float32)
    shared = {
        "a1": lhsA(False).astype(np.float16),
        "a2": lhsA(True).astype(np.float16),
        "b2w": lhsB().astype(np.float16),
        "c1w": lhsC(False).astype(np.float16),
        "c2w": lhsC(True).astype(np.float16),
        "chi": np.ascontiguousarray(chiv.reshape(M, 1)),
        "clo": np.ascontiguousarray((-chiv).reshape(M, 1)),
    }
    in_maps = [dict(shared, xin=Xin[c]) for c in range(NCORES)]
    return in_maps, c4


def _gather(results, c4):
    out = np.empty((B, T, H), np.float32)
    for c in range(NCORES):
        yo = np.asarray(results[c]["yout"], dtype=np.float32)  # [NBLK, KA, NB]
        full = yo.reshape(NBLK, TB, H, NB).transpose(3, 0, 1, 2).reshape(NB, TPAD, H)
        out[c * NB:(c + 1) * NB] = full[:, :T]
    if np.any(c4):
        out += c4
    return out


def kernel(**inputs):
    if "nc" not in _cache:
        _cache["nc"] = _build_program()
    in_maps, c4 = _prep_inputs(inputs)
    res = run_bass_kernel_spmd(_cache["nc"], in_maps, core_ids=list(range(NCORES)))
    return _gather(res.results, c4)


# revision 7
# speedup vs baseline: 1.3484x; 1.3346x over previous
"""v3: 2-pass variant.  Host ships u = enc(X) @ Win.  Per block:
  ps1 = A1'@u (+C1@car1)   A1' = sigma-col-scaled R^(j-k) triangle + carry cols
  s1m = min(ps1, chi)      DVE evict (carry rows pass via chi=1e30)
  v   = (s1m max clo) + u  DVE scalar_tensor_tensor, fp16 (= u + sigma*clip)
  ps2 = A2'@v (+C2@car2)   A2' = R^(j-k)@Dm triangle + carry cols
  s2  = copy(ps2)          ACT evict -> fp16 out + carry2
Carry matmuls lead their groups (start=True, strip 3).  DMAs grouped G=3
blocks to cut Sync-engine issue serialization.
"""

import os
import sys
from contextlib import ExitStack

for _p in ("/root/.axon_site/_ro/trn_rl_repo", "/opt/trn_rl_repo"):
    if os.path.isdir(_p) and _p not in sys.path:
        sys.path.append(_p)

import numpy as np
import ml_dtypes

import concourse.bass as bass
import concourse.tile as tile
from concourse import bacc, mybir
from concourse.bass_utils import run_bass_kernel_spmd

dt = mybir.dt
Alu = mybir.AluOpType
Act = mybir.ActivationFunctionType

B, T, NI, H = 8192, 512, 2, 10
NCORES = 8
NB = B // NCORES          # 1024
TB = 9
NBLK = 57
TPAD = TB * NBLK
KA = 10 * TB              # 90
CO = 96
M = CO + 10               # 106
NH = NB // 2              # 512
KS = 64
G = 3                     # blocks per DMA
NG = NBLK // G            # 19

_cache = {}


def _build_program():
    nc = bacc.Bacc("TRN2", target_bir_lowering=False, debug=False)
    f32, f16 = dt.float32, dt.float16
    uin = nc.dram_tensor("uin", [NG, KA, G * NB], f16, kind="ExternalInput").ap()
    da1 = nc.dram_tensor("a1", [KA, M], f16, kind="ExternalInput").ap()
    da2 = nc.dram_tensor("a2", [KA, M], f16, kind="ExternalInput").ap()
    dc1 = nc.dram_tensor("c1w", [10, M], f16, kind="ExternalInput").ap()
    dc2 = nc.dram_tensor("c2w", [10, M], f16, kind="ExternalInput").ap()
    dchi = nc.dram_tensor("chi", [M, 1], f32, kind="ExternalInput").ap()
    dclo = nc.dram_tensor("clo", [KA, 1], f32, kind="ExternalInput").ap()
    yout = nc.dram_tensor("yout", [NG, KA, G * NB], f16, kind="ExternalOutput").ap()

    with tile.TileContext(nc) as tc, ExitStack() as ctx:
        wp = ctx.enter_context(tc.tile_pool(name="weights", bufs=1))
        xp = ctx.enter_context(tc.tile_pool(name="uin", bufs=3))
        s1p = ctx.enter_context(tc.tile_pool(name="s1", bufs=4))
        vp = ctx.enter_context(tc.tile_pool(name="v", bufs=4))
        s2p = ctx.enter_context(tc.tile_pool(name="s2", bufs=3))
        sps = ctx.enter_context(tc.tile_pool(name="ps", bufs=4, space="PSUM"))

        a1 = wp.tile([KA, M], f16, tag="a1")
        nc.sync.dma_start(a1[:], da1[:])
        a2 = wp.tile([KA, M], f16, tag="a2")
        nc.sync.dma_start(a2[:], da2[:])
        c1 = wp.tile([M, M], f16, tag="c1")
        nc.sync.dma_start(c1[CO:M, :], dc1[:])
        c2 = wp.tile([M, M], f16, tag="c2")
        nc.sync.dma_start(c2[CO:M, :], dc2[:])
        chi = wp.tile([M, 1], f32, tag="chi")
        nc.sync.dma_start(chi[:], dchi[:])
        clo = wp.tile([KA, 1], f32, tag="clo")
        nc.sync.dma_start(clo[:], dclo[:])

        s1prev = s2prev = None
        for g in range(NG):
            ut = xp.tile([KA, G * NB], f16)
            nc.sync.dma_start(ut[:], uin[g])
            s2 = s2p.tile([M, G * NB], f16)
            for bi in range(G):
                o = bi * NB
                ps1 = sps.tile([M, NB], f32, tag="ps")
                for lo, hi in ((o, o + NH), (o + NH, o + NB)):
                    plo, phi = lo - o, hi - o
                    if not (g == 0 and bi == 0):
                        nc.tensor.matmul(ps1[:, plo:phi], c1[CO:M, :],
                                         s1prev[0][CO:M, s1prev[1] + plo:s1prev[1] + phi],
                                         start=True, stop=False,
                                         tile_position=(CO, 0))
                    nc.tensor.matmul(ps1[:, plo:phi], a1[:], ut[:, lo:hi],
                                     start=(g == 0 and bi == 0), stop=True)
                # ACT evicts ps1 plain (fp16); carry rows 96..105 ride along
                s1 = s1p.tile([M, NB], f16)
                nc.scalar.activation(s1[:, :NH], ps1[:, :NH], Act.Copy)
                nc.scalar.activation(s1[:, NH:], ps1[:, NH:], Act.Copy)
                # DVE: t = min(z~', chi) at 4x fp16, then v = max(t, clo) + u
                tt = vp.tile([KA, NB], f16)
                nc.vector.tensor_scalar(tt[:], s1[:KA, :], chi[:KA, :], None,
                                        Alu.min)
                v = vp.tile([KA, NB], f16)
                nc.vector.scalar_tensor_tensor(v[:], tt[:], clo[:],
                                               ut[:, o:o + NB], Alu.max, Alu.add)
                ps2 = sps.tile([M, NB], f32, tag="ps")
                for lo, hi in ((0, NH), (NH, NB)):
                    if not (g == 0 and bi == 0):
                        nc.tensor.matmul(ps2[:, lo:hi], c2[CO:M, :],
                                         s2prev[0][CO:M, s2prev[1] + lo:s2prev[1] + hi],
                                         start=True, stop=False,
                                         tile_position=(CO, 0))
                    nc.tensor.matmul(ps2[:, lo:hi], a2[:], v[:, lo:hi],
                                     start=(g == 0 and bi == 0), stop=True)
                # ps2 evict split across ACT (L) and DVE (R)
                nc.scalar.activation(s2[:, o:o + NH], ps2[:, :NH], Act.Copy)
                nc.vector.tensor_copy(s2[:, o + NH:o + NB], ps2[:, NH:])
                s1prev, s2prev = (s1, 0), (s2, o)
            nc.sync.dma_start(yout[g], s2[:KA, :])

    nc.compile()
    return nc


def _prep_inputs(inputs):
    X = np.ascontiguousarray(inputs["X"], dtype=np.float32)
    W1, b1v, W2, b2v = (np.asarray(inputs[k], np.float64) for k in ("W1", "b1", "W2", "b2"))
    Win, R, mbv = (np.asarray(inputs[k], np.float64) for k in ("Win", "R", "mb"))
    W3, b3v, W4, b4v = (np.asarray(inputs[k], np.float64) for k in ("W3", "b3", "W4", "b4"))
    Dm = W3 @ W4
    c4 = (b3v @ W4 + b4v).astype(np.float32)

    Rp = [np.eye(H)]
    for _ in range(TB + 1):
        Rp.append(Rp[-1] @ R)

    sig = np.where(mbv <= 0, -1.0, 1.0)
    Sg = np.diag(sig)
    absmb = np.abs(mbv)

    def lhsA1():
        L = np.zeros((KA, M), np.float64)
        for k in range(TB):
            for j in range(k, TB):
                L[10 * k:10 * k + 10, 10 * j:10 * j + 10] = Rp[j - k] @ Sg
            L[10 * k:10 * k + 10, CO:] = Rp[TB - 1 - k]
        return L

    def lhsA2():
        L = np.zeros((KA, M), np.float64)
        for k in range(TB):
            for j in range(k, TB):
                L[10 * k:10 * k + 10, 10 * j:10 * j + 10] = Rp[j - k] @ Dm
            L[10 * k:10 * k + 10, CO:] = Rp[TB - 1 - k]
        return L

    def lhsC(dec):
        L = np.zeros((10, M), np.float64)
        for j in range(TB):
            L[:, 10 * j:10 * j + 10] = (Rp[j + 1] @ Dm) if dec else (Rp[j + 1] @ Sg)
        L[:, CO:] = Rp[TB]
        return L

    # host encoder + input kernel (<1 GFLOP): u = relu(relu(X@W1+b1)@W2+b2)@Win
    x1 = np.maximum(X @ W1.astype(np.float32) + b1v.astype(np.float32), 0)
    x2 = np.maximum(x1 @ W2.astype(np.float32) + b2v.astype(np.float32), 0)
    u = x2 @ Win.astype(np.float32)
    Uc = u.reshape(NCORES, NB, T, H)
    Up = np.zeros((NCORES, NB, TPAD, H), np.float32)
    Up[:, :, :T] = Uc
    # [core, group, KA, G*NB]
    Uin = np.ascontiguousarray(
        Up.reshape(NCORES, NB, NG, G, TB, H).transpose(0, 2, 4, 5, 3, 1)
        .reshape(NCORES, NG, KA, G * NB).astype(np.float16)
    )

    chiv = np.concatenate([np.tile(absmb, TB), np.full(M - KA, 1e30)]).astype(np.float32)
    clov = (-np.tile(absmb, TB)).astype(np.float32)
    shared = {
        "a1": lhsA1().astype(np.float16),
        "a2": lhsA2().astype(np.float16),
        "c1w": lhsC(False).astype(np.float16),
        "c2w": lhsC(True).astype(np.float16),
        "chi": np.ascontiguousarray(chiv.reshape(M, 1)),
        "clo": np.ascontiguousarray(clov.reshape(KA, 1)),
    }
    in_maps = [dict(shared, uin=Uin[c]) for c in range(NCORES)]
    return in_maps, c4


def _gather(results, c4):
    out = np.empty((B, T, H), np.float32)
    for c in range(NCORES):
        yo = np.asarray(results[c]["yout"], dtype=np.float32)  # [NG, KA, G*NB]
        full = (yo.reshape(NG, TB, H, G, NB).transpose(4, 0, 3, 1, 2)
                .reshape(NB, TPAD, H))
        out[c * NB:(c + 1) * NB] = full[:, :T]
    if np.any(c4):
        out += c4
    return out


def kernel(**inputs):
    if "nc" not in _cache:
        _cache["nc"] = _build_program()
    in_maps, c4 = _prep_inputs(inputs)
    res = run_bass_kernel_spmd(_cache["nc"], in_maps, core_ids=list(range(NCORES)))
    return _gather(res.results, c4)


# revision 12
# speedup vs baseline: 1.3734x; 1.0185x over previous
"""Trainium2 Bass kernel for the Exprnn-style model (nn_Exprnn_2542620639651).

v5: 2-pass linear-scan decomposition with the carry contraction FOLDED into
the main matmuls.  Host ships u = enc(X)@Win (payload rows 0..89 of each
block tile); the 10-row carry state is copied (fp16 4x DVE, ~194ns/half)
into rows 96..105 of the next block's contraction tile, and the lhsT rows
96..105 hold the carry weights (R^(j+1) powers).  So each scan is ONE
K=106 matmul per 512-col PSUM bank:

  ps1 = A1ext @ [u; car1]      2 matmuls (bank halves)
  s1r = copy(ps1)              ACT evict fp16 (z~' rows + car1' rows)
  tt  = clip(s1r, +-|mb|)      DVE tensor_scalar 4x (= sigma*d, sign folded)
  v   = tt + u                 DVE tensor_tensor 2x
  ps2 = A2ext @ [v; car2]      2 matmuls
  s2r = copy(ps2)              ACT(+DVE tail) evict fp16 -> out + car2'

4 matmul slots/block (427ns fill each at the measured 1.2GHz fill rate),
PE/DVE/ACT all balanced at ~2us/block.  Rows 90..95 of contraction tiles
are dead (memset once per ring buffer).  DMAs grouped G=3 blocks.
Math validated in numpy sim (sim_v3.py) at 3.7e-3 max rel err.
"""

import os
import sys
from contextlib import ExitStack

for _p in ("/root/.axon_site/_ro/trn_rl_repo", "/opt/trn_rl_repo"):
    if os.path.isdir(_p) and _p not in sys.path:
        sys.path.append(_p)

import numpy as np
import ml_dtypes

import concourse.bass as bass
import concourse.tile as tile
from concourse import bacc, mybir
from concourse.bass_utils import run_bass_kernel_spmd

dt = mybir.dt
Alu = mybir.AluOpType
Act = mybir.ActivationFunctionType

B, T, NI, H = 8192, 512, 2, 10
NCORES = 8
NB = B // NCORES          # 1024
TB = 9
NBLK = 57
TPAD = TB * NBLK
KA = 10 * TB              # 90 payload rows
CO = 96                   # carry rows 96..105 (32-aligned base)
M = CO + 10               # 106: contraction K and psum output rows
NH = NB // 2              # 512
G = 3                     # blocks per DMA group
NG = NBLK // G            # 19
SP = NH + 256             # s2 evict split: ACT does [0:SP), DVE does [SP:NB)

_cache = {}


def _build_program():
    nc = bacc.Bacc("TRN2", target_bir_lowering=False, debug=False)
    f32, f16 = dt.float32, dt.float16
    uin = nc.dram_tensor("uin", [NG, KA, G * NB], f16, kind="ExternalInput").ap()
    da1 = nc.dram_tensor("a1", [M, M], f16, kind="ExternalInput").ap()
    da2 = nc.dram_tensor("a2", [M, M], f16, kind="ExternalInput").ap()
    dchi = nc.dram_tensor("chi", [KA, 1], f32, kind="ExternalInput").ap()
    dclo = nc.dram_tensor("clo", [KA, 1], f32, kind="ExternalInput").ap()
    yout = nc.dram_tensor("yout", [NG, KA, G * NB], f16, kind="ExternalOutput").ap()

    NU, NV, NS = 3, 4, 2   # ring sizes: u-group tiles, v tiles, s-group tiles

    with tile.TileContext(nc) as tc, ExitStack() as ctx:
        wp = ctx.enter_context(tc.tile_pool(name="weights", bufs=1))
        up = ctx.enter_context(tc.tile_pool(name="u", bufs=NU))
        vp = ctx.enter_context(tc.tile_pool(name="v", bufs=NV))
        tp = ctx.enter_context(tc.tile_pool(name="tt", bufs=3))
        s1p = ctx.enter_context(tc.tile_pool(name="s1", bufs=3))
        s2p = ctx.enter_context(tc.tile_pool(name="s2", bufs=NS))
        sps = ctx.enter_context(tc.tile_pool(name="ps", bufs=4, space="PSUM"))

        a1 = wp.tile([M, M], f16, tag="a1")
        nc.sync.dma_start(a1[:], da1[:])
        a2 = wp.tile([M, M], f16, tag="a2")
        nc.sync.dma_start(a2[:], da2[:])
        chi = wp.tile([KA, 1], f32, tag="chi")
        nc.sync.dma_start(chi[:], dchi[:])
        clo = wp.tile([KA, 1], f32, tag="clo")
        nc.sync.dma_start(clo[:], dclo[:])

        # rings: u/s2 are G-block-wide group tiles, v per-block
        uts = []
        for i in range(NU):
            t = up.tile([M, G * NB], f16, tag=f"u{i}")
            nc.gpsimd.memset(t[64:M, :], 0.0)   # dead rows + initial carry1=0
            uts.append(t)
        vts = []
        for i in range(NV):
            t = vp.tile([M, NB], f16, tag=f"v{i}")
            nc.gpsimd.memset(t[64:M, :], 0.0)
            vts.append(t)
        s2ts = []
        for i in range(NS):
            t = s2p.tile([M, G * NB], f16, tag=f"s2g{i}")
            s2ts.append(t)

        s1prev = None            # (tile, col offset) holding carry1 rows
        s2prev = None
        nc.sync.dma_start(uts[0][:KA, :], uin[0])
        for g in range(NG):
            ug = uts[g % NU]
            if g + 1 < NG:       # prefetch next group's input
                nc.sync.dma_start(uts[(g + 1) % NU][:KA, :], uin[g + 1])
            s2g = s2ts[g % NS]
            for bi in range(G):
                b = g * G + bi
                o = bi * NB
                # carry1 insert (fp16 4x, halves to shorten the chain)
                if b > 0:
                    nc.vector.tensor_copy(ug[CO:M, o:o + NH],
                                          s1prev[0][CO:M, s1prev[1]:s1prev[1] + NH])
                    nc.vector.tensor_copy(ug[CO:M, o + NH:o + NB],
                                          s1prev[0][CO:M, s1prev[1] + NH:s1prev[1] + NB])
                ps1 = sps.tile([M, NB], f32, tag="ps")
                nc.tensor.matmul(ps1[:, :NH], a1[:], ug[:, o:o + NH],
                                 start=True, stop=True)
                nc.tensor.matmul(ps1[:, NH:], a1[:], ug[:, o + NH:o + NB],
                                 start=True, stop=True)
                s1r = s1p.tile([M, NB], f16)
                nc.scalar.activation(s1r[:, :NH], ps1[:, :NH], Act.Copy)
                nc.scalar.activation(s1r[:, NH:], ps1[:, NH:], Act.Copy)

                tt = tp.tile([KA, NB], f16)
                nc.vector.tensor_scalar(tt[:], s1r[:KA, :], chi[:], clo[:],
                                        Alu.min, Alu.max)
                v = vts[b % NV]
                nc.vector.tensor_add(v[:KA, :], tt[:], ug[:KA, o:o + NB])
                if b > 0:
                    nc.vector.tensor_copy(v[CO:M, :NH],
                                          s2prev[0][CO:M, s2prev[1]:s2prev[1] + NH])
                    nc.vector.tensor_copy(v[CO:M, NH:],
                                          s2prev[0][CO:M, s2prev[1] + NH:s2prev[1] + NB])
                ps2 = sps.tile([M, NB], f32, tag="ps")
                nc.tensor.matmul(ps2[:, :NH], a2[:], v[:, :NH],
                                 start=True, stop=True)
                nc.tensor.matmul(ps2[:, NH:], a2[:], v[:, NH:],
                                 start=True, stop=True)
                nc.scalar.activation(s2g[:, o:o + SP], ps2[:, :SP], Act.Copy)
                nc.vector.tensor_copy(s2g[:, o + SP:o + NB], ps2[:, SP:])
                s1prev, s2prev = (s1r, 0), (s2g, o)
            nc.sync.dma_start(yout[g], s2g[:KA, :])

    nc.compile()
    return nc


def _prep_inputs(inputs):
    X = np.ascontiguousarray(inputs["X"], dtype=np.float32)
    W1, b1v, W2, b2v = (np.asarray(inputs[k], np.float64) for k in ("W1", "b1", "W2", "b2"))
    Win, R, mbv = (np.asarray(inputs[k], np.float64) for k in ("Win", "R", "mb"))
    W3, b3v, W4, b4v = (np.asarray(inputs[k], np.float64) for k in ("W3", "b3", "W4", "b4"))
    Dm = W3 @ W4
    c4 = (b3v @ W4 + b4v).astype(np.float32)

    Rp = [np.eye(H)]
    for _ in range(TB + 1):
        Rp.append(Rp[-1] @ R)

    sig = np.where(mbv <= 0, -1.0, 1.0)
    Sg = np.diag(sig)
    absmb = np.abs(mbv)

    def lhsExt(dec):
        # [M, M]: contraction rows 0..89 = u/v payload, 96..105 = carry-in;
        # output cols 0..89 = z~'/out, 96..105 = carry-out
        L = np.zeros((M, M), np.float64)
        for k in range(TB):
            for j in range(k, TB):
                blk = (Rp[j - k] @ Dm) if dec else (Rp[j - k] @ Sg)
                L[10 * k:10 * k + 10, 10 * j:10 * j + 10] = blk
            L[10 * k:10 * k + 10, CO:] = Rp[TB - 1 - k]
        for j in range(TB):
            blk = (Rp[j + 1] @ Dm) if dec else (Rp[j + 1] @ Sg)
            L[CO:, 10 * j:10 * j + 10] = blk
        L[CO:, CO:] = Rp[TB]
        return L

    # host encoder + input kernel (<1 GFLOP)
    x1 = np.maximum(X @ W1.astype(np.float32) + b1v.astype(np.float32), 0)
    x2 = np.maximum(x1 @ W2.astype(np.float32) + b2v.astype(np.float32), 0)
    u = x2 @ Win.astype(np.float32)
    Uc = u.reshape(NCORES, NB, T, H)
    Up = np.zeros((NCORES, NB, TPAD, H), np.float32)
    Up[:, :, :T] = Uc
    Uin = np.ascontiguousarray(
        Up.reshape(NCORES, NB, NG, G, TB, H).transpose(0, 2, 4, 5, 3, 1)
        .reshape(NCORES, NG, KA, G * NB).astype(np.float16)
    )

    chiv = np.tile(absmb, TB).astype(np.float32)
    shared = {
        "a1": lhsExt(False).astype(np.float16),
        "a2": lhsExt(True).astype(np.float16),
        "chi": np.ascontiguousarray(chiv.reshape(KA, 1)),
        "clo": np.ascontiguousarray((-chiv).reshape(KA, 1)),
    }
    in_maps = [dict(shared, uin=Uin[c]) for c in range(NCORES)]
    return in_maps, c4


def _gather(results, c4):
    out = np.empty((B, T, H), np.float32)
    for c in range(NCORES):
        yo = np.asarray(results[c]["yout"], dtype=np.float32)  # [NG, KA, G*NB]
        full = (yo.reshape(NG, TB, H, G, NB).transpose(4, 0, 3, 1, 2)
                .reshape(NB, TPAD, H))
        out[c * NB:(c + 1) * NB] = full[:, :T]
    if np.any(c4):
        out += c4
    return out


def kernel(**inputs):
    if "nc" not in _cache:
        _cache["nc"] = _build_program()
    in_maps, c4 = _prep_inputs(inputs)
    res = run_bass_kernel_spmd(_cache["nc"], in_maps, core_ids=list(range(NCORES)))
    return _gather(res.results, c4)


# revision 14
# speedup vs baseline: 2.2310x; 1.6245x over previous
"""Trainium2 Bass kernel for the Exprnn-style model (nn_Exprnn_2542620639651).

v6: 2-pass linear-scan decomposition, TB=11 timesteps/block (48 blocks),
carry contraction folded into the main matmuls, software-pipelined loop.

Per block (row layout: timestep j -> rows 10j (j<=8) / 106+10(j-9) (j=9,10),
carry rows 96..105, dead rows 90..95):
  ps1 = A1ext @ [u; car1]      2 matmuls (512-col PSUM banks), K=M=126
  s1r = copy(ps1)              ACT evict fp16 (z~' + new car1)
  tt  = clip(s1r, +-|mb|)      DVE tensor_scalar 4x (carry rows pass via 1e30)
  v   = tt + u                 DVE tensor_tensor 2x  (u carry rows are 0)
  car2 insert: v[96:106] <- s2 evict of prev block   (DVE fp16 copy)
  ps2 = A2ext @ [v; car2]      2 matmuls
  s2  = copy(ps2)              ACT [0:SP) + DVE [SP:) evict fp16
  car1 insert: next u tile[96:106] <- s1r[96:106]    (DVE fp16 copy)

The loop is rotated: iteration i emits scan2 of block i-1 BEFORE scan1 of
block i, so the PE never waits on the in-block evict->clip->add chain.
sigma (modrelu sign) is folded into A1ext's output columns; the decoder
Dm=W3@W4 into A2ext's.  Everything on device is fp16; PSUM fp32.
"""

import os
import sys
from contextlib import ExitStack

for _p in ("/root/.axon_site/_ro/trn_rl_repo", "/opt/trn_rl_repo"):
    if os.path.isdir(_p) and _p not in sys.path:
        sys.path.append(_p)

import numpy as np
import ml_dtypes

import concourse.bass as bass
import concourse.tile as tile
from concourse import bacc, mybir
from concourse.bass_utils import run_bass_kernel_spmd

dt = mybir.dt
Alu = mybir.AluOpType
Act = mybir.ActivationFunctionType

B, T, NI, H = 8192, 512, 2, 10
NCORES = 8
NB = B // NCORES          # 1024
TB = 11
NBLK = 48                 # 48*11 = 528 >= 512 (tail padded with zeros)
TPAD = TB * NBLK
KP = 10 * TB              # 110 payload rows
CO = 96                   # carry rows 96..105
M = 126                   # tile rows: 0..89 + dead 90..95 + carry + 106..125
NH = NB // 2              # 512
G = 3                     # blocks per DMA group
NG = NBLK // G            # 16
SP = 640                  # s2 evict split: ACT [0:SP), DVE [SP:NB)
PAYROWS = np.r_[0:90, 106:126]

_cache = {}


def _build_program():
    nc = bacc.Bacc("TRN2", target_bir_lowering=False, debug=False)
    f32, f16 = dt.float32, dt.float16
    uin = nc.dram_tensor("uin", [NG, M, G * NB], f16, kind="ExternalInput").ap()
    da1 = nc.dram_tensor("a1", [M, M], f16, kind="ExternalInput").ap()
    da2 = nc.dram_tensor("a2", [M, M], f16, kind="ExternalInput").ap()
    dchi = nc.dram_tensor("chi", [M, 1], f32, kind="ExternalInput").ap()
    dclo = nc.dram_tensor("clo", [M, 1], f32, kind="ExternalInput").ap()
    yout = nc.dram_tensor("yout", [NG, M, G * NB], f16, kind="ExternalOutput").ap()

    NU, NV, NS, NS1 = 3, 4, 2, 3

    with tile.TileContext(nc) as tc, ExitStack() as ctx:
        wp = ctx.enter_context(tc.tile_pool(name="weights", bufs=1))
        up = ctx.enter_context(tc.tile_pool(name="u", bufs=NU))
        vp = ctx.enter_context(tc.tile_pool(name="v", bufs=NV))
        tp = ctx.enter_context(tc.tile_pool(name="tt", bufs=3))
        s1p = ctx.enter_context(tc.tile_pool(name="s1", bufs=NS1))
        s2p = ctx.enter_context(tc.tile_pool(name="s2", bufs=NS))
        sps = ctx.enter_context(tc.tile_pool(name="ps", bufs=4, space="PSUM"))

        a1 = wp.tile([M, M], f16, tag="a1")
        nc.sync.dma_start(a1[:], da1[:])
        a2 = wp.tile([M, M], f16, tag="a2")
        nc.sync.dma_start(a2[:], da2[:])
        chi = wp.tile([M, 1], f32, tag="chi")
        nc.sync.dma_start(chi[:], dchi[:])
        clo = wp.tile([M, 1], f32, tag="clo")
        nc.sync.dma_start(clo[:], dclo[:])

        uts = []
        for i in range(NU):
            t = up.tile([M, G * NB], f16, tag=f"u{i}")
            uts.append(t)
        vts = []
        for i in range(NV):
            t = vp.tile([M, NB], f16, tag=f"v{i}")
            vts.append(t)
        s2ts = []
        for i in range(NS):
            t = s2p.tile([M, G * NB], f16, tag=f"s2g{i}")
            s2ts.append(t)

        s1rs = {}   # b -> s1r tile
        s2s = {}    # b -> (s2 group tile, col offset)

        def a1part(b):
            # scan1 of block b + v payload; v's carry rows completed later
            # by copy2 (emitted with a2part(b-1))
            g, bi = divmod(b, G)
            o = bi * NB
            ug = uts[g % NU]
            if bi == 0:
                nc.sync.dma_start(ug[:], uin[g])
                if g + 1 < NG:
                    nc.sync.dma_start(uts[(g + 1) % NU][:], uin[g + 1])
            if b > 0:
                nc.vector.tensor_copy(ug[CO:CO + 10, o:o + NB],
                                      s1rs.pop(b - 1)[CO:CO + 10, :])
            ps1 = sps.tile([M, NB], f32, tag="ps")
            nc.tensor.matmul(ps1[:, :NH], a1[:], ug[:, o:o + NH],
                             start=True, stop=True)
            nc.tensor.matmul(ps1[:, NH:], a1[:], ug[:, o + NH:o + NB],
                             start=True, stop=True)
            s1r = s1p.tile([M, NB], f16)
            nc.scalar.activation(s1r[:], ps1[:], Act.Copy)
            tt = tp.tile([M, NB], f16)
            nc.vector.tensor_scalar(tt[:], s1r[:], chi[:], clo[:],
                                    Alu.min, Alu.max)
            v = vts[b % NV]
            nc.vector.tensor_add(v[:], tt[:], ug[:, o:o + NB])
            if b == 0:
                nc.vector.memset(v[CO:CO + 10, :], 0.0)
            s1rs[b] = s1r

        def a2part(b):
            # scan2 of block b; afterwards emit copy2 completing v(b+1)
            g, bi = divmod(b, G)
            o = bi * NB
            v = vts[b % NV]
            s2g = s2ts[g % NS]
            ps2 = sps.tile([M, NB], f32, tag="ps")
            nc.tensor.matmul(ps2[:, :NH], a2[:], v[:, :NH],
                             start=True, stop=True)
            nc.tensor.matmul(ps2[:, NH:], a2[:], v[:, NH:],
                             start=True, stop=True)
            nc.scalar.activation(s2g[:, o:o + SP], ps2[:, :SP], Act.Copy)
            nc.vector.tensor_copy(s2g[:, o + SP:o + NB], ps2[:, SP:])
            if bi == G - 1:
                nc.sync.dma_start(yout[g], s2g[:])
            if b + 1 < NBLK:
                nc.vector.tensor_copy(vts[(b + 1) % NV][CO:CO + 10, :],
                                      s2g[CO:CO + 10, o:o + NB])

        for i in range(NBLK + 2):
            if i < NBLK:
                a1part(i)
            if i >= 2:
                a2part(i - 2)

    nc.compile()
    return nc


def _prep_inputs(inputs):
    X = np.ascontiguousarray(inputs["X"], dtype=np.float32)
    W1, b1v, W2, b2v = (np.asarray(inputs[k], np.float64) for k in ("W1", "b1", "W2", "b2"))
    Win, R, mbv = (np.asarray(inputs[k], np.float64) for k in ("Win", "R", "mb"))
    W3, b3v, W4, b4v = (np.asarray(inputs[k], np.float64) for k in ("W3", "b3", "W4", "b4"))
    Dm = W3 @ W4
    c4 = (b3v @ W4 + b4v).astype(np.float32)

    Rp = [np.eye(H)]
    for _ in range(TB + 1):
        Rp.append(Rp[-1] @ R)

    sig = np.where(mbv <= 0, -1.0, 1.0)
    Sg = np.diag(sig)
    absmb = np.abs(mbv)

    def rowOf(j):
        return 10 * j if j < 9 else 106 + 10 * (j - 9)

    def lhsExt(dec):
        L = np.zeros((M, M), np.float64)
        for k in range(TB):
            rk = rowOf(k)
            for j in range(k, TB):
                blk = (Rp[j - k] @ Dm) if dec else (Rp[j - k] @ Sg)
                L[rk:rk + 10, rowOf(j):rowOf(j) + 10] = blk
            L[rk:rk + 10, CO:CO + 10] = Rp[TB - 1 - k]
        for j in range(TB):
            blk = (Rp[j + 1] @ Dm) if dec else (Rp[j + 1] @ Sg)
            L[CO:CO + 10, rowOf(j):rowOf(j) + 10] = blk
        L[CO:CO + 10, CO:CO + 10] = Rp[TB]
        return L

    # host encoder + input kernel (<1 GFLOP)
    x1 = np.maximum(X @ W1.astype(np.float32) + b1v.astype(np.float32), 0)
    x2 = np.maximum(x1 @ W2.astype(np.float32) + b2v.astype(np.float32), 0)
    u = x2 @ Win.astype(np.float32)
    Uc = u.reshape(NCORES, NB, T, H)
    Up = np.zeros((NCORES, NB, TPAD, H), np.float32)
    Up[:, :, :T] = Uc
    # row-permuted per-block layout [NCORES, NG, M, G*NB]
    Ul = np.zeros((NCORES, NB, NBLK, M), np.float32)
    Ul[:, :, :, PAYROWS] = Up.reshape(NCORES, NB, NBLK, KP)
    Uin = np.ascontiguousarray(
        Ul.reshape(NCORES, NB, NG, G, M).transpose(0, 2, 4, 3, 1)
        .reshape(NCORES, NG, M, G * NB).astype(np.float16)
    )

    chiv = np.full(M, 1e30, np.float32)
    chiv[PAYROWS] = np.tile(absmb, TB).astype(np.float32)
    shared = {
        "a1": lhsExt(False).astype(np.float16),
        "a2": lhsExt(True).astype(np.float16),
        "chi": np.ascontiguousarray(chiv.reshape(M, 1)),
        "clo": np.ascontiguousarray((-chiv).reshape(M, 1)),
    }
    in_maps = [dict(shared, uin=Uin[c]) for c in range(NCORES)]
    return in_maps, c4


def _gather(results, c4):
    out = np.empty((B, T, H), np.float32)
    for c in range(NCORES):
        yo = np.asarray(results[c]["yout"], dtype=np.float32)  # [NG, M, G*NB]
        pay = yo[:, PAYROWS, :]                                # [NG, KP, G*NB]
        full = (pay.reshape(NG, TB, H, G, NB).transpose(4, 0, 3, 1, 2)
                .reshape(NB, TPAD, H))
        out[c * NB:(c + 1) * NB] = full[:, :T]
    if np.any(c4):
        out += c4
    return out


def kernel(**inputs):
    if "nc" not in _cache:
        _cache["nc"] = _build_program()
    in_maps, c4 = _prep_inputs(inputs)
    res = run_bass_kernel_spmd(_cache["nc"], in_maps, core_ids=list(range(NCORES)))
    return _gather(res.results, c4)
